# revision 1
# baseline (speedup 1.0000x reference)
"""Trainium2 Bass kernel for Luong-attention (nn_Attention_4174708212176).

out[b] = softmax(dec[b] @ (enc[b] @ W)^T) @ enc[b],  b = 0..7, one batch per core.

Precision scheme (validated on HW, end-to-end rel err 3.5e-4):
- M1 (enc @ W): hi/lo split-fp16, 3 accumulating PE passes (fp32-grade result).
- M2 (dec @ ep^T): fp16 hi*hi pass + the two hi/lo cross-terms in fp8e4m3
  with perf_mode=DoubleRow (0.5 cyc/row) and compensating 2^+-a host scales.
- M3 (P @ enc): plain fp16 (P is near-one-hot; 2^-11 error is negligible).
- Softmax fp32 on DVE (max, negated) + ACT (exp with accumulated row sums);
  1/sum is folded into the final PSUM->SBUF copy via activation(Copy, scale).
- P^T for M3 via PE transposes (fp16), 4 per PSUM bank, copied out on ACT/DVE.

The PE stream is software-pipelined: transposes+M3 of tile t-1 are emitted
after tile t's logits matmuls so the PE never waits on the softmax engines.

Layouts are prepared host-side: each core receives one packed fp16 tensor
(W hi/lo, encT hi/lo sc-major, decT hi, enc natural), DMA'd in segments so
M1 starts as soon as W and the first encT chunk arrive, plus one fp8 tensor
with the scaled dec hi/lo operands.
"""
import contextlib
import numpy as np

import concourse.bass as bass
import concourse.tile as tile
from concourse import bacc, mybir
from concourse.bass_utils import run_bass_kernel_spmd
from concourse.masks import make_identity

B, S, T, E, D = 8, 2048, 2048, 512, 512
P = 128
DO = D // P      # 4  d-tiles
EO = E // P      # 4  e-tiles
SO = S // P      # 16 s-tiles
TO = T // P      # 16 t-tiles
SC = S // 512    # 4  512-wide s-chunks
NCORES = 8

# packed free-dim offsets (fp16 elements per partition)
OFF_WH = 0                   # W hi      [4, 512]
OFF_WL = OFF_WH + EO * D
OFF_ETH = OFF_WL + EO * D    # encT hi   [4, 2048]
OFF_ETL = OFF_ETH + EO * S
OFF_DTH = OFF_ETL + EO * S   # decT hi   [4, 2048]
OFF_EN = OFF_DTH + DO * T    # enc natural [16, 512]
FREE = OFF_EN + SO * E

SEGS = [  # (name, offset, width)
    ("w", OFF_WH, 2 * EO * D),
    ("eth", OFF_ETH, EO * S),
    ("etl", OFF_ETL, EO * S),
    ("dt", OFF_DTH, DO * T),
    ("en", OFF_EN, SO * E),
]
FREE8 = 2 * DO * T           # d8h | d8l fp8e4

_compiled_nc = {}


def _build(reps=1):
    nc = bacc.Bacc()
    x_in = nc.declare_dram_parameter("x", [P, FREE], mybir.dt.float16, isOutput=False)
    x8_in = nc.declare_dram_parameter("x8", [P, FREE8], mybir.dt.float8e4, isOutput=False)
    out_d = nc.declare_dram_parameter("out", [T, E], mybir.dt.float32, isOutput=True)

    with tile.TileContext(nc) as tc:
        with tc.tile_pool(name="const", bufs=1) as cpool, \
             tc.tile_pool(name="ep", bufs=1) as eppool, \
             tc.tile_pool(name="work", bufs=3) as wpool, \
             tc.tile_pool(name="lbuf", bufs=2) as lpool, \
             tc.tile_pool(name="stat", bufs=4) as spool, \
             tc.tile_pool(name="psA", bufs=5, space="PSUM") as psA, \
             tc.tile_pool(name="psB", bufs=2, space="PSUM") as psB, \
             tc.tile_pool(name="psC", bufs=1, space="PSUM") as psC:

            ident = cpool.tile([P, P], mybir.dt.float16)
            make_identity(nc, ident[:])

            _ENGS = (mybir.EngineType.PE, mybir.EngineType.Activation,
                     mybir.EngineType.DVE, mybir.EngineType.SP,
                     mybir.EngineType.Pool)
            loop_ctx = (tc.For_i(0, reps, 1, hint_engines=_ENGS)
                        if reps > 1 else contextlib.nullcontext())
            with loop_ctx:
                _body(nc, tc, cpool, eppool, wpool, lpool, spool,
                      psA, psB, psC, x_in, x8_in, out_d, ident)

    nc.compile()
    return nc


def _body(nc, tc, cpool, eppool, wpool, lpool, spool, psA, psB, psC,
          x_in, x8_in, out_d, ident):
    seg = {}
    for name, off, width in SEGS:
        seg[name] = cpool.tile([P, width], mybir.dt.float16, tag=f"seg_{name}",
                               name=f"seg_{name}")
    segd = dict((n, (o, w)) for n, o, w in SEGS)
    nc.sync.dma_start(seg["w"][:], x_in.ap()[:, segd["w"][0]:segd["w"][0] + segd["w"][1]])
    # interleave encT hi/lo per-sc chunks so M1's split passes never wait
    for sc in range(SC):
        for name in ("eth", "etl"):
            off, width = segd[name]
            w4 = width // SC
            nc.sync.dma_start(seg[name][:, sc * w4:(sc + 1) * w4],
                              x_in.ap()[:, off + sc * w4:off + (sc + 1) * w4])
    for name in ("dt", "en"):
        off, width = segd[name]
        nc.sync.dma_start(seg[name][:], x_in.ap()[:, off:off + width])
    d8 = cpool.tile([P, DO, 2 * T], mybir.dt.float8e4, name="d8")
    nc.sync.dma_start(d8[:], x8_in.ap())

    def wh(eo, do):  # W hi tile [128, 128] (lhsT for M1)
        o = eo * D + do * P
        return seg["w"][:, o:o + P]

    def wl(eo, do):
        o = EO * D + eo * D + do * P
        return seg["w"][:, o:o + P]

    def eth(eo, sc):  # encT hi chunk [128, 512] (rhs for M1), sc-major
        o = sc * 4 * 512 + eo * 512
        return seg["eth"][:, o:o + 512]

    def etl(eo, sc):
        o = sc * 4 * 512 + eo * 512
        return seg["etl"][:, o:o + 512]

    def dth(do, tt):  # decT hi tile [128, 128] (lhsT for M2)
        o = do * T + tt * P
        return seg["dt"][:, o:o + P]

    def d8h_pair(dp, tt):  # [128, 2, 128] fp8 (DoubleRow lhsT)
        return d8[:, 2 * dp:2 * dp + 2, tt * P:tt * P + P]

    def d8l_pair(dp, tt):
        return d8[:, 2 * dp:2 * dp + 2, T + tt * P:T + tt * P + P]

    def encn(st):  # enc natural tile [128, 512] (rhs for M3)
        o = st * E
        return seg["en"][:, o:o + 512]

    # ---- M1: epT[d, s] = sum_e W[e, d] * encT[e, s], split-fp16 3 passes
    eph = eppool.tile([P, DO * S], mybir.dt.float16)  # [128, 4*2048]
    ep8l = eppool.tile([P, DO, S], mybir.dt.float8e4)  # (ep - eph) * 2^5
    ep8h = eppool.tile([P, DO, S], mybir.dt.float8e4)  # eph * 2^-7
    for do in range(DO):
        pss = [psA.tile([P, 512], mybir.dt.float32, tag="ps_l", name=f"m1_{do}_{sc}")
               for sc in range(SC)]
        i = 0
        for aw, ae in ((wh, eth), (wh, etl), (wl, eth)):
            for eo in range(EO):
                for sc in range(SC):
                    nc.tensor.matmul(pss[sc][:], aw(eo, do), ae(eo, sc),
                                     start=(i == 0), stop=(i == 3 * EO - 1),
                                     skip_group_check=True)
                i += 1
        for sc in range(SC):
            dst = slice(do * S + sc * 512, do * S + sc * 512 + 512)
            nc.scalar.copy(eph[:, dst], pss[sc][:])
            epl_t = wpool.tile([P, 512], mybir.dt.float16, name=f"epl_{do}_{sc}",
                               tag="epl_t")
            nc.vector.tensor_tensor(epl_t[:], pss[sc][:], eph[:, dst],
                                    mybir.AluOpType.subtract)
            nc.vector.tensor_scalar_mul(ep8l[:, do, sc * 512:(sc + 1) * 512],
                                        epl_t[:], float(2 ** 5))
            nc.scalar.mul(ep8h[:, do, sc * 512:(sc + 1) * 512], eph[:, dst],
                          float(2 ** -7))

    def ephc(do, sc):
        o = do * S + sc * 512
        return eph[:, o:o + 512]

    def ep8l_pair(dp, sc):  # [128, 2, 512] fp8 (DoubleRow rhs)
        return ep8l[:, 2 * dp:2 * dp + 2, sc * 512:(sc + 1) * 512]

    def ep8h_pair(dp, sc):
        return ep8h[:, 2 * dp:2 * dp + 2, sc * 512:(sc + 1) * 512]

    # ---- per t-tile: M2 logits -> softmax; transpose+M3 of the previous
    # tile are emitted after the next tile's M2 so the PE never waits on ACT.
    def emit_m2_softmax(tt):
        l_sb = lpool.tile([P, S], mybir.dt.float32, name=f"l{tt}", tag="l")
        pss = [psA.tile([P, 512], mybir.dt.float32, tag="ps_l", name=f"m2_{tt}_{sc}")
               for sc in range(SC)]
        for do in range(DO):
            for sc in range(SC):
                nc.tensor.matmul(pss[sc][:], dth(do, tt), ephc(do, sc),
                                 start=(do == 0), stop=False,
                                 skip_group_check=True)
        for ad, ae in ((d8h_pair, ep8l_pair), (d8l_pair, ep8h_pair)):
            last_pass = ae is ep8h_pair
            for dp in range(DO // 2):
                for sc in range(SC):
                    nc.tensor.matmul(pss[sc][:], ad(dp, tt), ae(dp, sc),
                                     start=False,
                                     stop=(last_pass and dp == DO // 2 - 1),
                                     perf_mode=mybir.MatmulPerfMode.DoubleRow,
                                     skip_group_check=True)
        for sc in range(SC):
            dst = slice(sc * 512, sc * 512 + 512)
            if sc % 2 == 0:
                nc.scalar.copy(l_sb[:, dst], pss[sc][:])
            else:
                nc.vector.tensor_copy(l_sb[:, dst], pss[sc][:])

        pmax = spool.tile([P, SC], mybir.dt.float32, name=f"pmax{tt}", tag="pmax")
        for sc in range(SC):
            nc.vector.tensor_reduce(pmax[:, sc:sc + 1], l_sb[:, sc * 512:(sc + 1) * 512],
                                    axis=mybir.AxisListType.X,
                                    op=mybir.AluOpType.max)
        negmax = spool.tile([P, 1], mybir.dt.float32, name=f"negmax{tt}", tag="negmax")
        nc.vector.tensor_reduce(negmax[:], pmax[:], axis=mybir.AxisListType.X,
                                op=mybir.AluOpType.max, negate=True)

        p_sb = wpool.tile([P, S], mybir.dt.float16, name=f"p{tt}", tag="p")
        sums = spool.tile([P, SC], mybir.dt.float32, name=f"sums{tt}", tag="sums")
        for sc in range(SC):
            nc.scalar.activation(p_sb[:, sc * 512:(sc + 1) * 512],
                                 l_sb[:, sc * 512:(sc + 1) * 512],
                                 mybir.ActivationFunctionType.Exp,
                                 bias=negmax[:], scale=1.0,
                                 accum_out=sums[:, sc:sc + 1])
        ssum = spool.tile([P, 1], mybir.dt.float32, name=f"ssum{tt}", tag="ssum")
        nc.vector.tensor_reduce(ssum[:], sums[:], axis=mybir.AxisListType.X,
                                op=mybir.AluOpType.add)
        recip = spool.tile([P, 1], mybir.dt.float32, name=f"recip{tt}", tag="recip")
        nc.vector.reciprocal(recip[:], ssum[:])
        return p_sb, recip

    def emit_tr_m3(tt, p_sb, recip):
        # transpose P [128t, 2048s] -> PT tiles [128s, 128t], batched 4 per PSUM
        pt_sb = wpool.tile([P, SO * P], mybir.dt.float16, name=f"pt{tt}", tag="pt")
        for q in range(SO // 4):
            ps_tr = psB.tile([P, 512], mybir.dt.float16, tag="ps_tr", name=f"tr{tt}_{q}")
            for j in range(4):
                st = q * 4 + j
                nc.tensor.transpose(ps_tr[:, j * P:(j + 1) * P],
                                    p_sb[:, st * P:(st + 1) * P], ident[:])
            dst = slice(q * 512, (q + 1) * 512)
            if q % 2 == 0:
                nc.scalar.copy(pt_sb[:, dst], ps_tr[:])
            else:
                nc.vector.tensor_copy(pt_sb[:, dst], ps_tr[:])

        # M3: out[t, e] = sum_s PT[s, t]^T * enc_n[s, e]
        ops = psC.tile([P, E], mybir.dt.float32, tag="ps_out", name=f"m3_{tt}")
        for st in range(SO):
            nc.tensor.matmul(ops[:], pt_sb[:, st * P:(st + 1) * P], encn(st),
                             start=(st == 0), stop=(st == SO - 1))
        out_sb = wpool.tile([P, E], mybir.dt.float32, name=f"o{tt}", tag="o")
        nc.scalar.activation(out_sb[:], ops[:],
                             mybir.ActivationFunctionType.Copy,
                             bias=0.0, scale=recip[:])
        nc.sync.dma_start(out_d.ap()[tt * P:(tt + 1) * P, :], out_sb[:])

    prev = None
    for tt in range(TO):
        cur = (tt, *emit_m2_softmax(tt))
        if prev is not None:
            emit_tr_m3(*prev)
        prev = cur
    emit_tr_m3(*prev)


def _part(x, ko):
    """[K, F] -> [128, ko, F] -> [128, ko*F] flat, partition = k % 128."""
    kf = x.reshape(ko, P, -1).transpose(1, 0, 2)
    return np.ascontiguousarray(kf.reshape(P, -1))


def _f16(x):
    return x.astype(np.float16)


def _f8(x):
    import ml_dtypes
    return x.astype(ml_dtypes.float8_e4m3fn)


def _pack_core(enc_b, dec_b, wseg):
    decT = np.ascontiguousarray(dec_b.T)          # [512, 2048]
    encT = np.ascontiguousarray(enc_b.T)          # [512, 2048]
    dth = _f16(decT)
    dtl32 = decT - dth.astype(np.float32)
    eth = _f16(encT)
    etl = _f16(encT - eth.astype(np.float32))
    en = _f16(enc_b)                              # [2048, 512]
    def scmajor(x):  # [128, EO*S] with [eo][sc][512] -> [sc][eo][512]
        v = _part(x, EO).reshape(P, EO, SC, 512)
        return np.ascontiguousarray(v.transpose(0, 2, 1, 3)).reshape(P, -1)

    segs = [
        wseg,
        scmajor(eth), scmajor(etl),
        _part(dth, DO),
        _part(en, SO),
    ]
    x = np.concatenate(segs, axis=1)
    d8h = _part(_f8(dth.astype(np.float32) * 2.0 ** -5), DO).reshape(P, DO, T)
    d8l = _part(_f8(dtl32 * 2.0 ** 7), DO).reshape(P, DO, T)
    x8 = np.ascontiguousarray(np.concatenate([d8h, d8l], axis=2)).reshape(P, -1)
    return x, x8


def _make_wseg(W):
    Wh = _f16(W)
    Wl_ = _f16(W - Wh.astype(np.float32))
    return np.concatenate([_part(Wh, EO), _part(Wl_, EO)], axis=1)


def kernel(enc_hidden_states, dec_hidden_states, W_att):
    enc = np.asarray(enc_hidden_states, np.float32)
    dec = np.asarray(dec_hidden_states, np.float32)
    W = np.asarray(W_att, np.float32)

    wseg = _make_wseg(W)
    packed = [_pack_core(enc[b], dec[b], wseg) for b in range(NCORES)]
    in_maps = [{"x": p[0], "x8": p[1]} for p in packed]

    if 1 not in _compiled_nc:
        _compiled_nc[1] = _build(1)

    res = run_bass_kernel_spmd(_compiled_nc[1], in_maps, list(range(NCORES)))
    out = np.stack([res.results[b]["out"] for b in range(NCORES)], axis=0)
    return out.astype(np.float32)


if __name__ == "__main__":
    rng = np.random.default_rng(0)
    enc = rng.standard_normal((B, S, E), dtype=np.float32)
    dec = rng.standard_normal((B, T, D), dtype=np.float32)
    W = rng.standard_normal((E, D), dtype=np.float32)
    out = kernel(enc, dec, W)
    print("out", out.shape, out.dtype)



# revision 2
# speedup vs baseline: 1.5731x; 1.5731x over previous
"""Trainium2 Bass kernel for Luong-attention (nn_Attention_4174708212176).

out[b] = softmax(dec[b] @ (enc[b] @ W)^T) @ enc[b],  b = 0..7, one batch per core.

Precision scheme (tolerance is rel 2e-2; this sits ~5e-3):
- All three matmuls in plain fp16 (PE upconverts to e10m11, fp32 PSUM accum).
  Logit noise eps ~ 0.3 from fp16 rounding of dec/ep shifts soft rows only;
  P is near-one-hot so the bulk of the output is unaffected.
- Softmax fp32 on DVE (max, negated) + ACT (exp with accumulated row sums);
  1/sum is folded into the final PSUM->SBUF copy via activation(Copy, scale).
- P^T for M3 via PE transposes (fp16), 4 per PSUM bank, copied out on ACT/DVE.

The PE stream is software-pipelined: transposes+M3 of tile t-1 are emitted
after tile t's logits matmuls so the PE never waits on the softmax engines.

Layouts are prepared host-side: each core receives one packed fp16 tensor
(W, encT sc-major, decT, enc natural), DMA'd in segments so M1 starts as
soon as W and the first encT chunk arrive.
"""
import contextlib
import numpy as np

import concourse.bass as bass
import concourse.tile as tile
from concourse import bacc, mybir
from concourse.bass_utils import run_bass_kernel_spmd
from concourse.masks import make_identity

B, S, T, E, D = 8, 2048, 2048, 512, 512
P = 128
DO = D // P      # 4  d-tiles
EO = E // P      # 4  e-tiles
SO = S // P      # 16 s-tiles
TO = T // P      # 16 t-tiles
SC = S // 512    # 4  512-wide s-chunks
NCORES = 8

# packed free-dim offsets (fp16 elements per partition)
OFF_WH = 0                   # W        [4, 512]
OFF_ETH = OFF_WH + EO * D    # encT     [4, 2048] sc-major
OFF_DTH = OFF_ETH + EO * S   # decT     [4, 2048]
OFF_EN = OFF_DTH + DO * T    # enc natural [16, 512]
FREE = OFF_EN + SO * E

SEGS = [  # (name, offset, width)
    ("w", OFF_WH, EO * D),
    ("eth", OFF_ETH, EO * S),
    ("dt", OFF_DTH, DO * T),
    ("en", OFF_EN, SO * E),
]

_compiled_nc = {}


def _build(reps=1):
    nc = bacc.Bacc()
    x_in = nc.declare_dram_parameter("x", [P, FREE], mybir.dt.float16, isOutput=False)
    out_d = nc.declare_dram_parameter("out", [T, E], mybir.dt.float32, isOutput=True)

    with tile.TileContext(nc) as tc:
        with tc.tile_pool(name="const", bufs=1) as cpool, \
             tc.tile_pool(name="ep", bufs=1) as eppool, \
             tc.tile_pool(name="work", bufs=3) as wpool, \
             tc.tile_pool(name="lbuf", bufs=2) as lpool, \
             tc.tile_pool(name="stat", bufs=4) as spool, \
             tc.tile_pool(name="psA", bufs=5, space="PSUM") as psA, \
             tc.tile_pool(name="psB", bufs=2, space="PSUM") as psB, \
             tc.tile_pool(name="psC", bufs=1, space="PSUM") as psC:

            ident = cpool.tile([P, P], mybir.dt.float16)
            make_identity(nc, ident[:])

            _ENGS = (mybir.EngineType.PE, mybir.EngineType.Activation,
                     mybir.EngineType.DVE, mybir.EngineType.SP,
                     mybir.EngineType.Pool)
            loop_ctx = (tc.For_i(0, reps, 1, hint_engines=_ENGS)
                        if reps > 1 else contextlib.nullcontext())
            with loop_ctx:
                _body(nc, tc, cpool, eppool, wpool, lpool, spool,
                      psA, psB, psC, x_in, out_d, ident)

    nc.compile()
    return nc


def _body(nc, tc, cpool, eppool, wpool, lpool, spool, psA, psB, psC,
          x_in, out_d, ident):
    seg = {}
    for name, off, width in SEGS:
        seg[name] = cpool.tile([P, width], mybir.dt.float16, tag=f"seg_{name}",
                               name=f"seg_{name}")
    segd = dict((n, (o, w)) for n, o, w in SEGS)
    nc.sync.dma_start(seg["w"][:], x_in.ap()[:, segd["w"][0]:segd["w"][0] + segd["w"][1]])
    # per-sc chunks of encT so M1's first accumulation group never waits
    for sc in range(SC):
        off, width = segd["eth"]
        w4 = width // SC
        nc.sync.dma_start(seg["eth"][:, sc * w4:(sc + 1) * w4],
                          x_in.ap()[:, off + sc * w4:off + (sc + 1) * w4])
    for name in ("dt", "en"):
        off, width = segd[name]
        nc.sync.dma_start(seg[name][:], x_in.ap()[:, off:off + width])

    def wh(eo, do):  # W tile [128, 128] (lhsT for M1)
        o = eo * D + do * P
        return seg["w"][:, o:o + P]

    def eth(eo, sc):  # encT chunk [128, 512] (rhs for M1), sc-major
        o = sc * 4 * 512 + eo * 512
        return seg["eth"][:, o:o + 512]

    def dth(do, tt):  # decT tile [128, 128] (lhsT for M2)
        o = do * T + tt * P
        return seg["dt"][:, o:o + P]

    def encn(st):  # enc natural tile [128, 512] (rhs for M3)
        o = st * E
        return seg["en"][:, o:o + 512]

    # ---- M1: epT[d, s] = sum_e W[e, d] * encT[e, s], fp16
    eph = eppool.tile([P, DO * S], mybir.dt.float16)  # [128, 4*2048]
    for do in range(DO):
        pss = [psA.tile([P, 512], mybir.dt.float32, tag="ps_l", name=f"m1_{do}_{sc}")
               for sc in range(SC)]
        for eo in range(EO):
            for sc in range(SC):
                nc.tensor.matmul(pss[sc][:], wh(eo, do), eth(eo, sc),
                                 start=(eo == 0), stop=(eo == EO - 1),
                                 skip_group_check=True)
        for sc in range(SC):
            dst = slice(do * S + sc * 512, do * S + sc * 512 + 512)
            if sc % 2 == 0:
                nc.scalar.copy(eph[:, dst], pss[sc][:])
            else:
                nc.vector.tensor_copy(eph[:, dst], pss[sc][:])

    def ephc(do, sc):
        o = do * S + sc * 512
        return eph[:, o:o + 512]

    # ---- per t-tile: M2 logits -> softmax; transpose+M3 of the previous
    # tile are emitted after the next tile's M2 so the PE never waits on ACT.
    def emit_m2_softmax(tt):
        l_sb = lpool.tile([P, S], mybir.dt.float32, name=f"l{tt}", tag="l")
        pss = [psA.tile([P, 512], mybir.dt.float32, tag="ps_l", name=f"m2_{tt}_{sc}")
               for sc in range(SC)]
        for do in range(DO):
            for sc in range(SC):
                nc.tensor.matmul(pss[sc][:], dth(do, tt), ephc(do, sc),
                                 start=(do == 0), stop=(do == DO - 1),
                                 skip_group_check=True)
        for sc in range(SC):
            dst = slice(sc * 512, sc * 512 + 512)
            if sc % 2 == 0:
                nc.scalar.copy(l_sb[:, dst], pss[sc][:])
            else:
                nc.vector.tensor_copy(l_sb[:, dst], pss[sc][:])

        pmax = spool.tile([P, SC], mybir.dt.float32, name=f"pmax{tt}", tag="pmax")
        for sc in range(SC):
            nc.vector.tensor_reduce(pmax[:, sc:sc + 1], l_sb[:, sc * 512:(sc + 1) * 512],
                                    axis=mybir.AxisListType.X,
                                    op=mybir.AluOpType.max)
        negmax = spool.tile([P, 1], mybir.dt.float32, name=f"negmax{tt}", tag="negmax")
        nc.vector.tensor_reduce(negmax[:], pmax[:], axis=mybir.AxisListType.X,
                                op=mybir.AluOpType.max, negate=True)

        p_sb = wpool.tile([P, S], mybir.dt.float16, name=f"p{tt}", tag="p")
        sums = spool.tile([P, SC], mybir.dt.float32, name=f"sums{tt}", tag="sums")
        for sc in range(SC):
            nc.scalar.activation(p_sb[:, sc * 512:(sc + 1) * 512],
                                 l_sb[:, sc * 512:(sc + 1) * 512],
                                 mybir.ActivationFunctionType.Exp,
                                 bias=negmax[:], scale=1.0,
                                 accum_out=sums[:, sc:sc + 1])
        ssum = spool.tile([P, 1], mybir.dt.float32, name=f"ssum{tt}", tag="ssum")
        nc.vector.tensor_reduce(ssum[:], sums[:], axis=mybir.AxisListType.X,
                                op=mybir.AluOpType.add)
        recip = spool.tile([P, 1], mybir.dt.float32, name=f"recip{tt}", tag="recip")
        nc.vector.reciprocal(recip[:], ssum[:])
        return p_sb, recip

    def emit_tr_m3(tt, p_sb, recip):
        # transpose P [128t, 2048s] -> PT tiles [128s, 128t], batched 4 per PSUM
        pt_sb = wpool.tile([P, SO * P], mybir.dt.float16, name=f"pt{tt}", tag="pt")
        for q in range(SO // 4):
            ps_tr = psB.tile([P, 512], mybir.dt.float16, tag="ps_tr", name=f"tr{tt}_{q}")
            for j in range(4):
                st = q * 4 + j
                nc.tensor.transpose(ps_tr[:, j * P:(j + 1) * P],
                                    p_sb[:, st * P:(st + 1) * P], ident[:])
            dst = slice(q * 512, (q + 1) * 512)
            if q % 2 == 0:
                nc.scalar.copy(pt_sb[:, dst], ps_tr[:])
            else:
                nc.vector.tensor_copy(pt_sb[:, dst], ps_tr[:])

        # M3: out[t, e] = sum_s PT[s, t]^T * enc_n[s, e]
        ops = psC.tile([P, E], mybir.dt.float32, tag="ps_out", name=f"m3_{tt}")
        for st in range(SO):
            nc.tensor.matmul(ops[:], pt_sb[:, st * P:(st + 1) * P], encn(st),
                             start=(st == 0), stop=(st == SO - 1))
        out_sb = wpool.tile([P, E], mybir.dt.float32, name=f"o{tt}", tag="o")
        nc.scalar.activation(out_sb[:], ops[:],
                             mybir.ActivationFunctionType.Copy,
                             bias=0.0, scale=recip[:])
        nc.sync.dma_start(out_d.ap()[tt * P:(tt + 1) * P, :], out_sb[:])

    prev = None
    for tt in range(TO):
        cur = (tt, *emit_m2_softmax(tt))
        if prev is not None:
            emit_tr_m3(*prev)
        prev = cur
    emit_tr_m3(*prev)


def _part(x, ko):
    """[K, F] -> [128, ko, F] -> [128, ko*F] flat, partition = k % 128."""
    kf = x.reshape(ko, P, -1).transpose(1, 0, 2)
    return np.ascontiguousarray(kf.reshape(P, -1))


def _f16(x):
    return x.astype(np.float16)


def _pack_core(enc_b, dec_b, wseg):
    decT = np.ascontiguousarray(dec_b.T)          # [512, 2048]
    encT = np.ascontiguousarray(enc_b.T)          # [512, 2048]
    dth = _f16(decT)
    eth = _f16(encT)
    en = _f16(enc_b)                              # [2048, 512]
    def scmajor(x):  # [128, EO*S] with [eo][sc][512] -> [sc][eo][512]
        v = _part(x, EO).reshape(P, EO, SC, 512)
        return np.ascontiguousarray(v.transpose(0, 2, 1, 3)).reshape(P, -1)

    segs = [
        wseg,
        scmajor(eth),
        _part(dth, DO),
        _part(en, SO),
    ]
    return np.concatenate(segs, axis=1)


def _make_wseg(W):
    return _part(_f16(W), EO)


def _pack_inputs(enc, dec, W):
    wseg = _make_wseg(W)
    return [{"x": _pack_core(enc[b], dec[b], wseg)} for b in range(NCORES)]


def kernel(enc_hidden_states, dec_hidden_states, W_att):
    enc = np.asarray(enc_hidden_states, np.float32)
    dec = np.asarray(dec_hidden_states, np.float32)
    W = np.asarray(W_att, np.float32)

    in_maps = _pack_inputs(enc, dec, W)

    if 1 not in _compiled_nc:
        _compiled_nc[1] = _build(1)

    res = run_bass_kernel_spmd(_compiled_nc[1], in_maps, list(range(NCORES)))
    out = np.stack([res.results[b]["out"] for b in range(NCORES)], axis=0)
    return out.astype(np.float32)


if __name__ == "__main__":
    rng = np.random.default_rng(0)
    enc = rng.standard_normal((B, S, E), dtype=np.float32)
    dec = rng.standard_normal((B, T, D), dtype=np.float32)
    W = rng.standard_normal((E, D), dtype=np.float32)
    out = kernel(enc, dec, W)
    print("out", out.shape, out.dtype)
